# revision 1
# baseline (speedup 1.0000x reference)
#!/usr/bin/env python3
"""TP-8 Trainium2 Bass kernel for a 4-layer Llama forward pass.

Model (hardcoded from the problem spec):
  H=2048, 32 q heads / 8 kv heads (GQA), head_dim 64, I=5632, L=4,
  V=32000, B=2, S=1024, rms eps 1e-5, neox rope theta 1e4, fp32 reference.

Sharding (vLLM-style tensor parallel over 8 cores):
  core r owns q heads 4r..4r+3, kv head r, gate/up columns and down rows
  for intermediate slice r*704..(r+1)*704, o_w rows for its 4 heads.
  Norm/residual work is replicated; AllReduce after o_proj and down_proj.

Layout: activations are feature-major x_T [feature, token] so every weight
matrix is directly usable as the matmul stationary operand (out = lhsT.T@rhs).
Matmuls run in float32r (full PE rate, ~1e-4 rel err). Cross-partition
reductions (rms stats, softmax denominators) go through the PE with
ones-vectors; softmax uses exp without max subtraction (logits are O(1) for
this model family) with the k->q transposed score layout so no P transpose
is needed; the ones-column appended to V yields the softmax denominators for
free inside the PV matmul.
"""
import os
import sys

sys.path.insert(0, '/opt/trn_rl_repo')

import numpy as np

# ---------------------------------------------------------------- constants
H = 2048
NH = 32
NKV = 8
HD = 64
I_FULL = 5632
L = 4
V = 32000
B, S = 2, 1024
T = B * S                     # 2048 tokens
EPS = 1e-5
THETA = 10000.0

NC_CORES = 8
QH = NH // NC_CORES           # 4 q heads per core
ISH = I_FULL // NC_CORES      # 704 intermediate slice
QCOLS = QH * HD               # 256
KT = H // 128                 # 16 k-tiles over H
CH = 512                      # token chunk (matmul N)
NCHUNK = T // CH              # 4
SUB = 256                     # norm-pass token subchunk
NSUB = CH // SUB              # 2
GU_M = 2 * ISH // 128         # 11 interleaved gate/up m-tiles
D_KT = 6                      # down k-tiles (5 full + 1 of 64)
SB_PER_B = S // 128           # 8 k-tiles of 128 tokens per batch
NEG = -1e9

_PROG_CACHE = {}


def _install_axon_trace_shim():
    """Register the NTFF profile hook that the container image is missing."""
    import types
    import antenv
    if getattr(antenv, 'axon_hooks', None) is not None:
        return
    try:
        from trn_agent_boot.trn_boot import _ntff_profile_via_ctypes
        hook = _ntff_profile_via_ctypes('/opt/axon/libaxon_pjrt.so')
    except Exception:
        hook = None
    mod = types.ModuleType('antenv.axon_hooks')
    mod.get_axon_ntff_profile_hook = lambda: hook
    mod.set_axon_ntff_profile_hook = lambda h: None
    sys.modules['antenv.axon_hooks'] = mod
    antenv.axon_hooks = mod


class _PhaseStop(Exception):
    pass


def _build_program():
    import concourse.bass as bass
    import concourse.bacc as bacc
    import concourse.tile as tile
    import concourse.mybir as mybir
    from concourse.masks import make_identity

    dt = mybir.dt
    F32 = dt.float32
    F32R = dt.float32r
    AF = mybir.ActivationFunctionType
    ALU = mybir.AluOpType

    nc = bacc.Bacc("TRN2", target_bir_lowering=False, debug=False,
                   enable_asserts=False, num_devices=NC_CORES)

    # ------------------------------------------------------------- inputs
    emb_ap = nc.dram_tensor("emb", [V, H], F32, kind="ExternalInput").ap()
    ids_ap = nc.dram_tensor("ids", [T, 1], dt.int32, kind="ExternalInput").ap()
    wqk_ap = nc.dram_tensor("wqk", [L, H, QCOLS + HD], F32R, kind="ExternalInput").ap()
    wv_ap = nc.dram_tensor("wv", [L, H, HD], F32R, kind="ExternalInput").ap()
    wo_ap = nc.dram_tensor("wo", [L, QCOLS, H], F32R, kind="ExternalInput").ap()
    wgil_ap = nc.dram_tensor("wgil", [L, H, 2 * ISH], F32R, kind="ExternalInput").ap()
    wd_ap = nc.dram_tensor("wd", [L, ISH, H], F32R, kind="ExternalInput").ap()
    cos_ap = nc.dram_tensor("cosr", [128, T], F32, kind="ExternalInput").ap()
    sin_ap = nc.dram_tensor("sinr", [128, T], F32, kind="ExternalInput").ap()
    mask_ap = nc.dram_tensor("masks", [128, 4, CH], F32, kind="ExternalInput").ap()
    perm_ap = nc.dram_tensor("perm", [128, 128], F32R, kind="ExternalInput").ap()
    nw_ap = nc.dram_tensor("normw", [H, 1], F32R, kind="ExternalInput").ap()
    out_ap = nc.dram_tensor("out", [T, H], F32, kind="ExternalOutput").ap()

    from contextlib import ExitStack
    with tile.TileContext(nc) as tc, ExitStack() as ctx:
        dram = ctx.enter_context(tc.tile_pool(name="dram", bufs=1, space="DRAM"))
        h_dram = dram.tile([H, T], F32, tag="h_buf")
        aro_ins = [dram.tile([H, T], F32, tag=f"aro_in{l}", name=f"aro_in{l}")
                   for l in range(L)]
        aro_outs = [dram.tile([H, T], F32, tag=f"aro_out{l}", name=f"aro_out{l}",
                              addr_space="Shared") for l in range(L)]
        ard_ins = [dram.tile([H, T], F32, tag=f"ard_in{l}", name=f"ard_in{l}")
                   for l in range(L)]
        ard_outs = [dram.tile([H, T], F32, tag=f"ard_out{l}", name=f"ard_out{l}",
                              addr_space="Shared") for l in range(L)]
        sb_const = ctx.enter_context(tc.tile_pool(name="const", bufs=1))
        sb_w = ctx.enter_context(tc.tile_pool(name="w", bufs=1))
        sb_wstream = ctx.enter_context(tc.tile_pool(name="ws", bufs=2))
        sb_act = ctx.enter_context(tc.tile_pool(name="act", bufs=1))
        sb_small = ctx.enter_context(tc.tile_pool(name="small", bufs=4))
        sb_stage = ctx.enter_context(tc.tile_pool(name="stage", bufs=4))
        ps_mm = ctx.enter_context(tc.tile_pool(name="psmm", bufs=2, space="PSUM"))
        ps_sc = ctx.enter_context(tc.tile_pool(name="pssc", bufs=2, space="PSUM"))
        ps_at = ctx.enter_context(tc.tile_pool(name="psat", bufs=1, space="PSUM"))
        ps_aux = ctx.enter_context(tc.tile_pool(name="psaux", bufs=2, space="PSUM"))

        # ---------------------------------------------------- constants
        ident32 = sb_const.tile([128, 128], F32)
        make_identity(nc, ident32[:])
        identr = sb_const.tile([128, 128], F32R)
        nc.vector.tensor_copy(identr[:], ident32[:])
        ones_col = sb_const.tile([128, 1], F32R)
        nc.any.memset(ones_col[:].bitcast(F32), 1.0)
        ones_row = sb_const.tile([1, 128], F32R)
        nc.any.memset(ones_row[:].bitcast(F32), 1.0)
        zero_b = sb_const.tile([128, 1], F32)
        nc.any.memset(zero_b[:], 0.0)
        cos_t = sb_const.tile([128, T], F32)
        nc.sync.dma_start(cos_t[:], cos_ap[:])
        sin_t = sb_const.tile([128, T], F32)
        nc.sync.dma_start(sin_t[:], sin_ap[:])
        mask_t = sb_const.tile([128, 4, CH], F32)
        nc.sync.dma_start(mask_t[:], mask_ap[:])
        perm_t = sb_const.tile([128, 128], F32R)
        nc.sync.dma_start(perm_t[:], perm_ap[:])
        nw_t = sb_const.tile([128, KT, 1], F32R)
        nc.sync.dma_start(nw_t[:], nw_ap.rearrange("(kt p) o -> p kt o", p=128))

        # ---------------------------------------------------- embedding
        # h_dram[:, :] = emb[ids].T   (feature-major), replicated on all cores
        for g in range(T // 128):
            ids_t = sb_small.tile([128, 1], dt.int32, tag="ids", bufs=2)
            nc.sync.dma_start(ids_t[:], ids_ap[g * 128:(g + 1) * 128, :])
            tok = sb_wstream.tile([128, H], F32, tag="gil")  # shares gil slot
            nc.gpsimd.indirect_dma_start(
                out=tok[:], out_offset=None, in_=emb_ap[:],
                in_offset=bass.IndirectOffsetOnAxis(ap=ids_t[:, :1], axis=0))
            for hc in range(KT):
                tp = ps_aux.tile([128, 128], F32, tag="aux")
                nc.tensor.transpose(out=tp[:], in_=tok[:, hc * 128:(hc + 1) * 128],
                                    identity=ident32[:])
                st = sb_stage.tile([128, 128], F32, tag="stage")
                nc.vector.tensor_copy(st[:], tp[:])
                nc.sync.dma_start(
                    h_dram[hc * 128:(hc + 1) * 128, g * 128:(g + 1) * 128], st[:])

        # ------------------------------------------------- per-layer weights
        def load_layer_weights(l):
            wqk_t = sb_w.tile([128, KT, QCOLS + HD], F32R, tag="wqk")
            nc.sync.dma_start(wqk_t[:], wqk_ap[l].rearrange("(kt p) m -> p kt m", p=128))
            wv_t = sb_w.tile([128, KT, HD], F32R, tag="wv")
            nc.sync.dma_start(wv_t[:], wv_ap[l].rearrange("(kt p) m -> p kt m", p=128))
            return wqk_t, wv_t

        # ---------------------------------------------------- norm pass
        def norm_chunk(n, ar_src, xhat_tiles, final=False):
            """h (+= ar_src) -> rms-normalize chunk n -> xhat tiles (f32r),
            or (final) transpose+scale straight to the output tensor."""
            if True:
                for sub in range(NSUB):
                    t0 = n * CH + sub * SUB
                    tsl = slice(t0, t0 + SUB)
                    hts = []
                    stats = ps_aux.tile([1, SUB], F32, tag="aux")
                    for kt in range(KT):
                        fsl = slice(kt * 128, (kt + 1) * 128)
                        ht = sb_act.tile([128, SUB], F32, tag="ht",
                                         bufs=KT + 1)
                        nc.sync.dma_start(ht[:], h_dram[fsl, tsl])
                        if ar_src is not None:
                            art = sb_small.tile([128, SUB], F32, tag="art", bufs=2)
                            nc.sync.dma_start(art[:], ar_src[fsl, tsl])
                            nc.vector.tensor_tensor(out=ht[:], in0=ht[:],
                                                    in1=art[:], op=ALU.add)
                            if not final:
                                nc.sync.dma_start(h_dram[fsl, tsl], ht[:])
                        hts.append(ht)
                        x2 = sb_small.tile([128, SUB], F32R, tag="x2", bufs=2)
                        nc.scalar.activation(x2[:], ht[:], AF.Square,
                                             bias=zero_b[:], scale=1.0)
                        nc.tensor.matmul(stats[:], ones_col[:], x2[:],
                                         start=(kt == 0), stop=(kt == KT - 1))
                    # s = 1/sqrt(mean + eps)
                    mrow = sb_small.tile([1, SUB], F32, tag="mrow", bufs=2)
                    nc.vector.tensor_scalar(out=mrow[:], in0=stats[:],
                                            scalar1=1.0 / H, scalar2=EPS,
                                            op0=ALU.mult, op1=ALU.add)
                    sq = sb_small.tile([1, SUB], F32, tag="sq", bufs=2)
                    nc.scalar.activation(sq[:], mrow[:], AF.Sqrt,
                                         bias=zero_b[:1, :], scale=1.0)
                    srow = sb_small.tile([1, SUB], F32R, tag="srow", bufs=2)
                    with nc.allow_low_precision("f32r bits are f32; rounding happens at matmul read"):
                        nc.vector.reciprocal(srow[:], sq[:])
                    sbc = ps_aux.tile([128, SUB], F32, tag="aux")
                    nc.tensor.matmul(sbc[:], ones_row[:], srow[:],
                                     start=True, stop=True)
                    if not final:
                        for kt in range(KT):
                            xh = xhat_tiles[(n, kt)]
                            nc.vector.tensor_tensor(
                                out=xh[:, sub * SUB:(sub + 1) * SUB],
                                in0=hts[kt][:], in1=sbc[:], op=ALU.mult)
                    else:
                        for kt in range(KT):
                            xf = sb_small.tile([128, SUB], F32, tag="xf", bufs=2)
                            nc.vector.scalar_tensor_tensor(
                                out=xf[:], in0=hts[kt][:], scalar=nw_t[:, kt],
                                in1=sbc[:], op0=ALU.mult, op1=ALU.mult)
                            for u in range(SUB // 128):
                                tp = ps_aux.tile([128, 128], F32, tag="aux")
                                nc.tensor.transpose(
                                    out=tp[:], in_=xf[:, u * 128:(u + 1) * 128],
                                    identity=ident32[:])
                                st = sb_stage.tile([128, 128], F32, tag="stage")
                                nc.vector.tensor_copy(st[:], tp[:])
                                nc.sync.dma_start(
                                    out_ap[t0 + u * 128:t0 + (u + 1) * 128,
                                           kt * 128:(kt + 1) * 128], st[:])

        # ------------------------------------------- qkv + rope + v (chunk)
        def qkv_chunk(l, n, wqk_t, wv_t, xhat_tiles, qc_tiles, khat, vtok):
            tsl = slice(n * CH, (n + 1) * CH)
            # q (2 head-pair m-tiles) and k (64-col m-tile)
            for m in range(3):
                mp = 128 if m < 2 else 64
                csl = slice(m * 128, m * 128 + mp)
                ps = ps_mm.tile([mp, CH], F32, tag="mm")
                for kt in range(KT):
                    nc.tensor.matmul(ps[:], wqk_t[:, kt, csl],
                                     xhat_tiles[(n, kt)][:],
                                     start=(kt == 0), stop=(kt == KT - 1))
                # rope: out = x*cos + swap(x)*sin_signed
                qs = sb_small.tile([mp, CH], F32R, tag="qs", bufs=2)
                nc.scalar.activation(qs[:], ps[:], AF.Copy, bias=0.0, scale=1.0)
                swp = ps_aux.tile([mp, CH], F32, tag="aux")
                nc.tensor.matmul(swp[:], perm_t[:mp, :mp], qs[:],
                                 start=True, stop=True)
                t1 = sb_small.tile([mp, CH], F32, tag="t1", bufs=2)
                nc.vector.tensor_tensor(out=t1[:], in0=qs[:],
                                        in1=cos_t[:mp, tsl], op=ALU.mult)
                dst = qc_tiles[(n, m)][:] if m < 2 else khat[:64, tsl]
                nc.vector.tensor_tensor(out=dst, in0=swp[:],
                                        in1=sin_t[:mp, tsl], op=ALU.mult)
                nc.vector.tensor_tensor(out=dst, in0=dst, in1=t1[:], op=ALU.add)
                if m == 2:
                    nc.vector.tensor_copy(khat[64:, tsl], khat[:64, tsl])
            # v: feature-major matmul then transpose to token-major
            psv = ps_mm.tile([HD, CH], F32, tag="mm")
            for kt in range(KT):
                nc.tensor.matmul(psv[:], wv_t[:, kt, :], xhat_tiles[(n, kt)][:],
                                 start=(kt == 0), stop=(kt == KT - 1))
            vfm = sb_small.tile([HD, CH], F32R, tag="vfm", bufs=2)
            nc.scalar.activation(vfm[:], psv[:], AF.Copy, bias=0.0, scale=1.0)
            for w in range(CH // 128):
                g = n * (CH // 128) + w
                tp = ps_aux.tile([128, HD], F32R, tag="aux")
                nc.tensor.transpose(out=tp[:], in_=vfm[:, w * 128:(w + 1) * 128],
                                    identity=identr[:HD, :HD])
                nc.vector.tensor_copy(vtok[:, g, :HD], tp[:])
                nc.any.memset(vtok[:, g, HD:HD + 1].bitcast(F32), 1.0)

        # ------------------------------------------------- attention chunk
        def attn_chunk(l, c, qc_tiles, khat, vtok, attn_c):
            b, j = divmod(c, 2)
            for qh in range(QH):
                pair, odd = divmod(qh, 2)
                base = odd * 64
                at = ps_at.tile([HD + 1, CH], F32, tag="at")
                band = list(range(0, 4 * (j + 1)))
                for i in band:
                    gi = SB_PER_B * b + i          # global 128-token k-tile
                    d = i - 4 * j
                    sc = ps_sc.tile([128, CH], F32, tag="sc")
                    nc.tensor.matmul(
                        sc[:], khat[base:base + 64, gi * 128:(gi + 1) * 128],
                        qc_tiles[(c, pair)][base:base + 64, :],
                        start=True, stop=True)
                    if d >= 0:
                        nc.vector.tensor_tensor(out=sc[:], in0=sc[:],
                                                in1=mask_t[:, d, :], op=ALU.add)
                    pt = sb_small.tile([128, CH], F32R, tag="pt", bufs=2)
                    nc.scalar.activation(pt[:], sc[:], AF.Exp,
                                         bias=zero_b[:], scale=float(HD ** -0.5))
                    nc.tensor.matmul(at[:], vtok[:, gi, :], pt[:],
                                     start=(i == band[0]), stop=(i == band[-1]))
                rrow = sb_small.tile([1, CH], F32R, tag="rrow", bufs=2)
                with nc.allow_low_precision("f32r bits are f32; rounding happens at matmul read"):
                    nc.vector.reciprocal(rrow[:], at[HD:HD + 1, :])
                rbc = ps_aux.tile([HD, CH], F32, tag="aux")
                nc.tensor.matmul(rbc[:], ones_row[:, :HD], rrow[:],
                                 start=True, stop=True)
                rbs = sb_small.tile([HD, CH], F32, tag="rbs", bufs=2)
                nc.vector.tensor_copy(rbs[:], rbc[:])
                nc.vector.tensor_tensor(out=attn_c[pair][base:base + 64, :],
                                        in0=at[:HD, :], in1=rbs[:], op=ALU.mult)

        # --------------------------------------------------- o-proj chunk
        def o_chunk(l, c, attn_c, aro_in):
            tsl = slice(c * CH, (c + 1) * CH)
            for m in range(KT):
                wom = sb_wstream.tile([128, 2, 128], F32R, tag="wo", bufs=3)
                nc.sync.dma_start(
                    wom[:], wo_ap[l][:, m * 128:(m + 1) * 128]
                    .rearrange("(kt p) m -> p kt m", p=128))
                ps = ps_mm.tile([128, CH], F32, tag="mm")
                for kt in range(2):
                    nc.tensor.matmul(ps[:], wom[:, kt, :],
                                     attn_c[kt][:], start=(kt == 0), stop=(kt == 1))
                st = sb_stage.tile([128, CH], F32, tag="ost", bufs=2)
                nc.scalar.activation(st[:], ps[:], AF.Copy, bias=0.0, scale=1.0)
                nc.sync.dma_start(aro_in[m * 128:(m + 1) * 128, tsl], st[:])

        # ------------------------------------------------------ ffn chunk
        def ffn_chunk(l, n, xhat_tiles, ffn_tiles, ard_in):
            # gate_up (interleaved 64-blocks) + silu*up
            for m in range(GU_M):
                wg = sb_wstream.tile([128, KT, 128], F32R, tag="gil")
                nc.sync.dma_start(
                    wg[:], wgil_ap[l][:, m * 128:(m + 1) * 128]
                    .rearrange("(kt p) m -> p kt m", p=128))
                ps = ps_mm.tile([128, CH], F32, tag="mm")
                for kt in range(KT):
                    nc.tensor.matmul(ps[:], wg[:, kt, :], xhat_tiles[(n, kt)][:],
                                     start=(kt == 0), stop=(kt == KT - 1))
                sg = sb_small.tile([64, CH], F32, tag="sg", bufs=2)
                nc.scalar.activation(sg[:], ps[:64, :], AF.Silu,
                                     bias=zero_b[:64, :], scale=1.0)
                fkt, fhalf = divmod(m, 2)
                nc.vector.tensor_tensor(
                    out=ffn_tiles[fkt][fhalf * 64:fhalf * 64 + 64, :],
                    in0=sg[:], in1=ps[64:, :], op=ALU.mult)
            # down
            tsl = slice(n * CH, (n + 1) * CH)
            for m in range(KT):
                wdm = sb_wstream.tile([128, D_KT, 128], F32R, tag="wd")
                nc.sync.dma_start(
                    wdm[:, :D_KT - 1, :],
                    wd_ap[l][:640, m * 128:(m + 1) * 128]
                    .rearrange("(kt p) m -> p kt m", p=128))
                nc.sync.dma_start(wdm[:64, D_KT - 1, :],
                                  wd_ap[l][640:, m * 128:(m + 1) * 128])
                ps = ps_mm.tile([128, CH], F32, tag="mm")
                for kt in range(D_KT):
                    kp = 128 if kt < D_KT - 1 else 64
                    nc.tensor.matmul(ps[:], wdm[:kp, kt, :],
                                     ffn_tiles[kt][:kp, :],
                                     start=(kt == 0), stop=(kt == D_KT - 1))
                st = sb_stage.tile([128, CH], F32, tag="ost", bufs=2)
                nc.scalar.activation(st[:], ps[:], AF.Copy, bias=0.0, scale=1.0)
                nc.sync.dma_start(ard_in[m * 128:(m + 1) * 128, tsl], st[:])

        # --------------------------------------------------------- layers
        KSTOP = os.environ.get('KSTOP', 'full')

        def stop_after(phase):
            if KSTOP == phase:
                raise _PhaseStop()

        rg = [list(range(NC_CORES))]
        ar_src = None
        try:
            _emit_layers = True
            stop_after('embed')
        except _PhaseStop:
            _emit_layers = False
        if _emit_layers:
          try:
            for l in range(L):
              wqk_t, wv_t = load_layer_weights(l)
              khat = sb_act.tile([128, T], F32R, tag="khat")
              vtok = sb_act.tile([128, T // 128, HD + 1], F32R, tag="vtok")
              xhat_tiles = {(n, kt): sb_act.tile([128, CH], F32R, tag="xhat",
                                                 bufs=KT + 2, name=f"xh{n}_{kt}")
                            for n in range(NCHUNK) for kt in range(KT)}
              qc_tiles = {(n, m): sb_small.tile([128, CH], F32R, tag="qc", bufs=2,
                                                name=f"qc{n}_{m}")
                          for n in range(NCHUNK) for m in range(2)}
              for n in range(NCHUNK):
                  norm_chunk(n, ar_src, xhat_tiles)
                  stop_after('norm')
                  attn_c = [sb_small.tile([128, CH], F32R, tag="atc", bufs=2,
                                          name=f"atc{n}_{i2}")
                            for i2 in range(2)]
                  qkv_chunk(l, n, wqk_t, wv_t, xhat_tiles, qc_tiles, khat, vtok)
                  stop_after('qkv')
                  attn_chunk(l, n, qc_tiles, khat, vtok, attn_c)
                  stop_after('attn')
                  o_chunk(l, n, attn_c, aro_ins[l])
                  stop_after('o')
              nc.gpsimd.collective_compute(
                  "AllReduce", mybir.AluOpType.add, replica_groups=rg,
                  ins=[aro_ins[l].opt()], outs=[aro_outs[l].opt()])
              stop_after('aro')
              xhat2 = {(n, kt): sb_act.tile([128, CH], F32R, tag="xhat",
                                            bufs=KT + 2, name=f"xh2_{n}_{kt}")
                       for n in range(NCHUNK) for kt in range(KT)}
              for n in range(NCHUNK):
                  norm_chunk(n, aro_outs[l], xhat2)
                  ffn_tiles = [sb_small.tile([128, CH], F32R, tag="ffn",
                                             bufs=D_KT, name=f"ffn{n}_{i2}")
                               for i2 in range(D_KT)]
                  ffn_chunk(l, n, xhat2, ffn_tiles, ard_ins[l])
                  stop_after('ffn')
              nc.gpsimd.collective_compute(
                  "AllReduce", mybir.AluOpType.add, replica_groups=rg,
                  ins=[ard_ins[l].opt()], outs=[ard_outs[l].opt()])
              ar_src = ard_outs[l]
              stop_after('ard')
              stop_after('layer0')
          except _PhaseStop:
            pass
        for n in range(NCHUNK):
            norm_chunk(n, ar_src, None, final=True)

    nc.compile()
    return nc


def _prep_inputs(inputs):
    """Host-side sharding + constant prep. Returns per-core in_maps."""
    ids = np.ascontiguousarray(
        np.asarray(inputs['input_ids'], dtype=np.int32).reshape(T, 1))
    emb = np.ascontiguousarray(np.asarray(inputs['embed_w'], dtype=np.float32))
    qkv_w = np.asarray(inputs['qkv_w'], dtype=np.float32)
    o_w = np.asarray(inputs['o_w'], dtype=np.float32)
    gu_w = np.asarray(inputs['gate_up_w'], dtype=np.float32)
    dn_w = np.asarray(inputs['down_w'], dtype=np.float32)
    ln1 = np.asarray(inputs['ln1_w'], dtype=np.float32)
    ln2 = np.asarray(inputs['ln2_w'], dtype=np.float32)
    nw = np.asarray(inputs['norm_w'], dtype=np.float32)
    pos = np.asarray(inputs['positions'], dtype=np.float32).reshape(T)

    # rope tables: row r uses inv_freq[r % 32]; sign flips for first half of
    # each 64-row (=head) block; rows repeat every 64 so one [128, T] table
    # serves the 2-head-per-tile layout.
    half = HD // 2
    invf = 1.0 / (THETA ** (np.arange(half, dtype=np.float32) / half))
    r = np.arange(128)
    ang = pos[None, :] * invf[r % half][:, None]          # [128, T]
    cosr = np.cos(ang).astype(np.float32)
    sgn = np.where((r % HD) < half, -1.0, 1.0).astype(np.float32)
    sinr = (np.sin(ang) * sgn[:, None]).astype(np.float32)

    # additive causal masks for the diagonal band: keep k<=q
    kk = np.arange(128)[:, None]
    qq = np.arange(CH)[None, :]
    masks = np.stack([np.where(128 * d + kk <= qq, 0.0, NEG)
                      for d in range(4)], axis=1).astype(np.float32)  # [128,4,CH]

    permm = np.zeros((128, 128), np.float32)
    permm[np.arange(128) ^ 32, np.arange(128)] = 1.0

    # fold ln weights into the consuming projections (rows scaled over H)
    qkv_f = qkv_w * ln1[:, :, None]
    gu_f = gu_w * ln2[:, :, None]

    in_maps = []
    for rcore in range(NC_CORES):
        qsl = slice(rcore * QCOLS, (rcore + 1) * QCOLS)
        ksl = slice(NH * HD + rcore * HD, NH * HD + (rcore + 1) * HD)
        vsl = slice((NH + NKV) * HD + rcore * HD, (NH + NKV) * HD + (rcore + 1) * HD)
        wqk = np.concatenate([qkv_f[:, :, qsl], qkv_f[:, :, ksl]], axis=2)
        wv = qkv_f[:, :, vsl]
        wo = o_w[:, rcore * QCOLS:(rcore + 1) * QCOLS, :]
        gate = gu_f[:, :, rcore * ISH:(rcore + 1) * ISH]
        up = gu_f[:, :, I_FULL + rcore * ISH:I_FULL + (rcore + 1) * ISH]
        wgil = np.empty((L, H, 2 * ISH), np.float32)
        for j in range(GU_M):
            wgil[:, :, j * 128:j * 128 + 64] = gate[:, :, j * 64:(j + 1) * 64]
            wgil[:, :, j * 128 + 64:(j + 1) * 128] = up[:, :, j * 64:(j + 1) * 64]
        wd = dn_w[:, rcore * ISH:(rcore + 1) * ISH, :]
        in_maps.append({
            'emb': emb, 'ids': ids,
            'wqk': np.ascontiguousarray(wqk),
            'wv': np.ascontiguousarray(wv),
            'wo': np.ascontiguousarray(wo),
            'wgil': np.ascontiguousarray(wgil),
            'wd': np.ascontiguousarray(wd),
            'cosr': cosr, 'sinr': sinr, 'masks': masks, 'perm': permm,
            'normw': np.ascontiguousarray(nw.reshape(H, 1)),
        })
    return in_maps


def _get_program():
    if 'prog' not in _PROG_CACHE:
        _install_axon_trace_shim()
        _PROG_CACHE['prog'] = _build_program()
    return _PROG_CACHE['prog']


def _run(inputs, trace=False):
    from concourse.bass_utils import run_bass_kernel_spmd
    import concourse.bass_utils as bass_utils
    bass_utils.upload_artifacts = lambda tmpdir: "(skipped)"
    nc = _get_program()
    in_maps = _prep_inputs(inputs)
    res = run_bass_kernel_spmd(nc, in_maps, core_ids=list(range(NC_CORES)),
                               trace=trace)
    out = np.asarray(res.results[0]['out']).reshape(B, S, H)
    return out, res


def kernel(**inputs):
    out, _ = _run(inputs, trace=False)
    return out


def kernel_traced(**inputs):
    out, res = _run(inputs, trace=True)
    return out, res


if __name__ == '__main__':
    if os.environ.get('KBUILD'):
        _install_axon_trace_shim()
        _build_program()
        print("BUILD OK")

